# revision 28
# baseline (speedup 1.0000x reference)
"""Trainium2 Bass kernel for the BiDAF-style attention-flow layer (v2).

S[b,t,j] = H.w_h + U.w_u + (H*w_hu).U + bias
c2q      = softmax_j(S) @ U
q2c      = softmax_t(max_j S) @ H   (broadcast over t)
out      = concat([H, c2q, H*c2q, H*q2c], axis=-1)

Sharding: data-parallel over batch B=64 across 8 NeuronCores (8 batches per
core); W/b replicated; no collectives.

v2 design notes (vs the 111us v1):
 - All DRAM layouts are partition-major so every load/store is one fat
   contiguous descriptor per partition (4-12KB) instead of 0.5-2KB packets.
 - One exp activation per batch over [65, 1024]; all weight prep (sU+b,
   w_hu*U^T stationaries) is batched once up front.
 - Per-output-segment work is balanced across engines: seg0 (c2q/Z) split
   between DVE tensor_scalar and scalar activations, seg1 on DVE, seg2 on
   gpsimd with a 0-stride broadcast AP.
 - A warm-up matmul burst at kernel start keeps the PE HAM clock-gate at
   2.4GHz (v1 ran every matmul at the cold 1.2GHz rate).
 - Optional XBAR_HB mode re-creates the natural-layout H on chip from the
   d-major copy with DMA xbar transposes, saving 4.2MB/core of HBM reads.
"""

import numpy as np
import ml_dtypes

import concourse.bass as bass
import concourse.mybir as mybir
import concourse.tile as tile
from concourse.bass_utils import run_bass_kernel_spmd
from concourse.masks import make_identity

B, T, J, D = 64, 1024, 64, 256
NCORES = 8
BL = B // NCORES  # batches per core
NT = T // 128     # t-tiles per batch
F32 = mybir.dt.float32
BF16 = mybir.dt.bfloat16
AX = mybir.AxisListType.X
AF = mybir.ActivationFunctionType
MUL = mybir.AluOpType.mult
ADD = mybir.AluOpType.add
MAX = mybir.AluOpType.max

XBAR_HB = False   # build natural-layout H on chip via DMA xbar transposes
WARMUP_MM = 16    # N=512 PE warm-up matmuls (~7us cold) to flip the HAM gate
SEG0_DVE = 2      # pairs 0..SEG0_DVE-1 scale on DVE, rest on scalar ACT


def bcast(ap, n, axis=1):
    """Insert a 0-stride axis of length n into an AP (free-dim broadcast)."""
    new = list(ap.ap)
    new.insert(axis, [0, n])
    return bass.AP(tensor=ap.tensor, offset=ap.offset, ap=new)


def split_multi_waits(nc, max_waits=1):
    """Walrus in this container rejects instructions with more than a couple
    of embedded sync waits. Hoist extras into standalone EventSemaphore
    instructions right before the offending instruction."""
    n = 0
    for fn in nc.m.functions:
        for bb in fn.blocks:
            new_insts = []
            for inst in bb.instructions:
                si = getattr(inst, "sync_info", None)
                if si is not None and si.on_wait and len(si.on_wait) > max_waits:
                    waits = list(si.on_wait)
                    for w in waits[:-max_waits]:
                        n += 1
                        ev = mybir.InstEventSemaphore(
                            name=f"I-wsplit-{n}", ins=[], outs=[]
                        )
                        ev.engine = inst.engine
                        ev.sync_info = mybir.SyncInfo(on_wait=[w], on_update=[])
                        new_insts.append(ev)
                    inst.sync_info = mybir.SyncInfo(
                        on_wait=waits[-max_waits:], on_update=list(si.on_update)
                    )
                new_insts.append(inst)
            bb.instructions[:] = new_insts
    return n


def build_nc():
    nc = bass.Bass()
    HTR = nc.declare_dram_parameter("HTR", [BL, 128, 2, T], BF16, isOutput=False)
    if not XBAR_HB:
        HbR = nc.declare_dram_parameter("HbR", [BL, 128, NT, D], BF16,
                                        isOutput=False)
    Ub = nc.declare_dram_parameter("Ub", [BL, J, D], BF16, isOutput=False)
    UTb = nc.declare_dram_parameter("UTb", [128, BL, 2, J], BF16, isOutput=False)
    W = nc.declare_dram_parameter("W", [3 * D], F32, isOutput=False)
    b = nc.declare_dram_parameter("b", [1], F32, isOutput=False)
    out = nc.declare_dram_parameter("out", [BL, 128, NT, 3 * D], BF16,
                                    isOutput=True)

    with tile.TileContext(nc) as tc:
        with (
            tc.tile_pool(name="singles", bufs=1) as singles,
            tc.tile_pool(name="htp", bufs=3) as htp,
            tc.tile_pool(name="hbp", bufs=5) as hbp,
            tc.tile_pool(name="etp", bufs=2) as etp,
            tc.tile_pool(name="outp", bufs=4) as outp,
            tc.tile_pool(name="small", bufs=3) as small,
            # PSUM: 2 + 1 + 3 + 2 = 8 banks
            tc.tile_pool(name="ps_s", bufs=1, space="PSUM") as ps_s,
            tc.tile_pool(name="ps_et", bufs=1, space="PSUM") as ps_et,
            tc.tile_pool(name="ps_c", bufs=3, space="PSUM") as ps_c,
            tc.tile_pool(name="ps_q", bufs=1, space="PSUM") as ps_q,
        ):
            # ---------------- one-time setup ------------------------------
            # U/W loads go out FIRST (scalar queue) since the S stationaries
            # derive from them; the first H batches follow on sync. The sU
            # chain (ub_all, w_u_bc, b_col) loads before the rhs_w chain.
            ub_all = singles.tile([J, BL, D], BF16)
            nc.scalar.dma_start(
                out=ub_all[:], in_=Ub.rearrange("n j d -> j n d")
            )
            w_u_bc = singles.tile([J, D], F32)
            wsl = W[D : 2 * D]
            nc.scalar.dma_start(
                out=w_u_bc[:],
                in_=bass.AP(tensor=wsl.tensor, offset=wsl.offset,
                            ap=[[0, J]] + list(wsl.ap)),
            )
            b_col = singles.tile([J, 1], F32)
            bsl = b[0:1]
            nc.scalar.dma_start(
                out=b_col[:],
                in_=bass.AP(tensor=bsl.tensor, offset=bsl.offset,
                            ap=[[0, J]] + list(bsl.ap)),
            )
            ut_all = singles.tile([128, BL, 2, J], BF16)
            nc.scalar.dma_start(out=ut_all[:], in_=UTb[:])
            whu_col = singles.tile([128, 2], F32)
            wh_col = singles.tile([128, 2], F32)
            nc.scalar.dma_start(
                out=whu_col[:, :],
                in_=W[2 * D : 3 * D].rearrange("(k p) -> p k", p=128),
            )
            nc.scalar.dma_start(
                out=wh_col[:, :],
                in_=W[0:D].rearrange("(k p) -> p k", p=128),
            )

            def load_batch(bi):
                ht = htp.tile([128, 2, T], BF16, tag="ht")
                nc.sync.dma_start(out=ht[:], in_=HTR[bi])
                hb = hbp.tile([128, NT, D], BF16, tag="hb")
                if XBAR_HB:
                    for n in range(NT):
                        for c in range(2):
                            nc.sync.dma_start(
                                out=hb[:, n, 128 * c : 128 * (c + 1)],
                                in_=ht[:, c, 128 * n : 128 * (n + 1)],
                                transpose=True,
                            )
                else:
                    nc.sync.dma_start(out=hb[:], in_=HbR[bi])
                return ht, hb

            cur = load_batch(0)
            nxt = load_batch(1)

            ident_bf = singles.tile([128, 128], BF16)
            make_identity(nc, ident_bf[:])
            ones_row_bf = singles.tile([1, 128], BF16)
            nc.vector.memset(ones_row_bf[:], 1.0)
            ones_col_bf = singles.tile([128, 1], BF16)
            nc.vector.memset(ones_col_bf[:], 1.0)

            # PE warm-up: junk matmuls (N=512 via a 0-stride repeat of the
            # identity) into the S psum bank while the first DMAs land. Needs
            # >2 HAM windows (~7us) of sustained activity to reliably flip
            # the clock gate to 2.4GHz before real work starts.
            stp_warm = ps_q.tile([65, 512], F32, tag="qb")
            for i in range(WARMUP_MM):
                nc.tensor.matmul(
                    stp_warm[:], ident_bf[:, 0:65],
                    bcast(ident_bf[:], 4), start=True, stop=True,
                    skip_group_check=True,
                )

            # sU + b for all batches; row 64 stays 0 so the batched exp also
            # produces exp(sH) in its last row.
            su_scr = singles.tile([J, BL, D], F32)
            nc.gpsimd.tensor_mul(
                su_scr[:], ub_all[:], bcast(w_u_bc[:], BL)
            )
            su_raw = singles.tile([J, BL], F32)
            nc.vector.reduce_sum(su_raw[:], su_scr[:], axis=AX, op=ADD)
            su_ext = singles.tile([J + 1, BL], F32)
            nc.vector.tensor_scalar_add(su_ext[0:J, :], su_raw[:], b_col[:])
            nc.vector.memset(su_ext[J : J + 1, :], 0.0)

            # stationary weights [w_hu*U^T | w_h] for all batches
            rhs_w = singles.tile([128, BL, 2, J + 1], BF16)
            for c in range(2):
                nc.vector.tensor_scalar_mul(
                    rhs_w[:, :, c, 0:J], ut_all[:, :, c, :],
                    whu_col[:, c : c + 1],
                )
                nc.vector.tensor_copy(
                    rhs_w[:, :, c, J : J + 1],
                    bcast(wh_col[:, c : c + 1], BL),
                )

            # ---------------- per-batch pipeline --------------------------
            # The q2c tail for batch bi runs two batches later so none of
            # its six engine hops sits on the critical path.
            # pending state: [hb, ot3, em, em_s, bi, q2czt]
            pend1 = None
            pend2 = None

            def emit_q2czt(st):
                hb_, ot3_, em_, em_s_, tbi = st[:5]
                q2czt_ = ps_q.tile([1, D + 8], F32, tag="qz")
                for ti in range(NT):
                    nc.tensor.matmul(
                        q2czt_[0:1, 0:D], em_[:, ti : ti + 1], hb_[:, ti, :],
                        start=(ti == 0), stop=(ti == NT - 1),
                        skip_group_check=True,
                    )
                nc.tensor.matmul(q2czt_[0:1, D : D + 1], em_s_[:],
                                 ones_col_bf[:], start=True, stop=True,
                                 skip_group_check=True)
                st.append(q2czt_)

            def emit_tail_compute(st):
                hb_, ot3_, em_, em_s_, tbi, q2czt_ = st
                # q2c = (sum_t em*H) / (sum_t em)
                ztinv = small.tile([1, 1], F32, tag="ztinv")
                nc.vector.reciprocal(ztinv[:], q2czt_[0:1, D : D + 1])
                q2c_row = small.tile([1, D], BF16, tag="q2crow")
                nc.scalar.activation(q2c_row[:], q2czt_[0:1, 0:D], AF.Copy,
                                     scale=ztinv[:])
                q2cbp = ps_q.tile([128, D], F32, tag="qb")
                nc.tensor.matmul(q2cbp[:], ones_row_bf[:], q2c_row[:],
                                 start=True, stop=True)
                q2cb = small.tile([128, D], BF16, tag="q2cb")
                nc.scalar.copy(q2cb[:], q2cbp[:])
                # seg2 = H * q2c (broadcast over t-tiles) on gpsimd, keeping
                # the DVE free for the seg0/seg1/stats stream it governs
                for hh in range(2):
                    sl = slice(4 * hh, 4 * (hh + 1))
                    nc.gpsimd.tensor_mul(
                        ot3_[:, sl, 2 * D : 3 * D], hb_[:, sl, :],
                        bcast(q2cb[:], 4),
                    )

            def emit_store(st):
                ot3_, tbi = st[1], st[4]
                if tbi >= BL - 2:
                    # endgame stores split in halves across both queues
                    for hh in range(2):
                        sl = slice(4 * hh, 4 * (hh + 1))
                        q = nc.sync if (hh + tbi) % 2 == 0 else nc.scalar
                        q.dma_start(out=out[tbi, :, sl], in_=ot3_[:, sl])
                else:
                    nc.sync.dma_start(out=out[tbi], in_=ot3_[:])

            for bi in range(BL):
                ht, hb = cur

                # ---- S^T: stp[j|sH, t] --------------------------------
                stp = ps_s.tile([65, 2, 512], F32, tag="stp")
                for h in range(2):
                    for c in range(2):
                        nc.tensor.matmul(
                            stp[:, h, :], rhs_w[:, bi, c, :],
                            ht[:, c, 512 * h : 512 * (h + 1)],
                            start=(c == 0), stop=(c == 1),
                        )
                    # exp half h while half 1-h is still on the PE
                    if h == 0:
                        et = etp.tile([J + 1, T], BF16, tag="et")
                    nc.scalar.activation(
                        et[:, 512 * h : 512 * (h + 1)], stp[:, h, :], AF.Exp,
                        bias=su_ext[:, bi : bi + 1], scale=1.0,
                    )

                if bi + 2 <= BL - 1:
                    cur, nxt = nxt, load_batch(bi + 2)
                else:
                    cur = nxt

                # the lag-2 batch's q2c accumulation fills the PE while the
                # scalar engine computes this batch's exp.
                if pend2 is not None:
                    emit_q2czt(pend2)

                # ---- c2q matmuls + stats transposes --------------------
                ot3 = outp.tile([128, NT, 3 * D], BF16, tag="ot")
                etT = ps_et.tile([128, NT, 80], BF16, tag="etT")
                cqs = []
                for p in range(NT // 2):
                    cq = ps_c.tile([128, 2, D], F32, tag="cq")
                    for k in range(2):
                        ti = 2 * p + k
                        nc.tensor.matmul(
                            cq[:, k, :],
                            et[0:J, 128 * ti : 128 * (ti + 1)],
                            ub_all[:, bi, :], start=True, stop=True,
                        )
                        nc.tensor.transpose(
                            etT[:, ti, 0 : J + 1],
                            et[:, 128 * ti : 128 * (ti + 1)],
                            ident_bf[0 : J + 1, 0 : J + 1],
                        )
                    cqs.append(cq)

                if pend2 is not None:
                    emit_tail_compute(pend2)

                # ---- softmax stats ------------------------------------
                r = small.tile([128, NT], BF16, tag="r")
                nc.vector.reduce_max(r[:], etT[:, :, 0:J], axis=AX, op=MAX)
                zs = small.tile([128, NT], F32, tag="zs")
                nc.vector.reduce_sum(zs[:], etT[:, :, 0:J], axis=AX, op=ADD)
                zinv = small.tile([128, NT], F32, tag="zinv")
                nc.vector.reciprocal(zinv[:], zs[:])
                # em = exp(sH) * max_j E; its per-partition sums feed the
                # deferred Zt matmul two batches later.
                em = small.tile([128, NT], BF16, tag="em")
                nc.vector.tensor_tensor(
                    out=em[:], in0=etT[:, :, J], in1=r[:], op=MUL
                )
                em_sb = small.tile([128, 1], BF16, tag="emsb")
                with nc.allow_low_precision(
                    reason="DVE reduces in f32 internally; one bf16 round"
                ):
                    nc.vector.reduce_sum(em_sb[:], em[:], axis=AX, op=ADD)

                # ---- normalize c2q (split DVE/scalar) + seg1 -----------
                for p in range(NT // 2):
                    cq = cqs[p]
                    if p < SEG0_DVE:
                        # one op per pair: zinv broadcast along d via a
                        # 0-stride axis ([128, 2, 256] * [128, 2, (0)256])
                        zsl = zinv[:, 2 * p : 2 * p + 2]
                        nc.vector.tensor_tensor(
                            out=ot3[:, 2 * p : 2 * p + 2, 0:D], in0=cq[:],
                            in1=bcast(zsl, D, axis=2), op=MUL,
                        )
                    else:
                        for k in range(2):
                            ti = 2 * p + k
                            nc.scalar.activation(
                                ot3[:, ti, 0:D], cq[:, k, :], AF.Copy,
                                scale=zinv[:, ti : ti + 1],
                            )
                # seg1 = seg0 * H in one wide bf16 op
                nc.vector.tensor_tensor(
                    out=ot3[:, :, D : 2 * D], in0=ot3[:, :, 0:D],
                    in1=hb[:], op=MUL,
                )

                if pend2 is not None:
                    emit_store(pend2)
                pend2 = pend1
                pend1 = [hb, ot3, em, em_sb, bi]

            # endgame: interleave the last two batches' tails so their
            # engine chains overlap instead of running serially.
            emit_q2czt(pend2)
            emit_q2czt(pend1)
            emit_tail_compute(pend2)
            emit_tail_compute(pend1)
            emit_store(pend2)
            emit_store(pend1)

    split_multi_waits(nc)
    return nc


_NC_CACHE = None


def get_nc():
    global _NC_CACHE
    if _NC_CACHE is None:
        _NC_CACHE = build_nc()
    return _NC_CACHE


def make_in_maps(H, U, W, b):
    """Shard + lay out inputs (host does dtype rounding + transposes only)."""
    H = np.ascontiguousarray(np.asarray(H, dtype=np.float32))
    U = np.ascontiguousarray(np.asarray(U, dtype=np.float32))
    W = np.ascontiguousarray(np.asarray(W, dtype=np.float32))
    b = np.ascontiguousarray(np.asarray(b, dtype=np.float32))
    Hb = H.astype(ml_dtypes.bfloat16)
    Ubf = U.astype(ml_dtypes.bfloat16)
    # d-major H: [B, 128(q), 2(c), T] with d = c*128 + q
    HTR = np.ascontiguousarray(
        Hb.reshape(B, T, 2, 128).transpose(0, 3, 2, 1)
    )
    # natural H, partition-major: [B, 128(p), NT, D] with t = n*128 + p
    HbR = np.ascontiguousarray(
        Hb.reshape(B, NT, 128, D).transpose(0, 2, 1, 3)
    )
    # d-major U: [128(q), B, 2(c), J]
    UTb = np.ascontiguousarray(
        Ubf.reshape(B, J, 2, 128).transpose(3, 0, 2, 1)
    )
    maps = []
    for i in range(NCORES):
        sl = slice(i * BL, (i + 1) * BL)
        m = {
            "HTR": HTR[sl],
            "Ub": Ubf[sl],
            "UTb": UTb[:, sl],
            "W": W,
            "b": b,
        }
        if not XBAR_HB:
            m["HbR"] = HbR[sl]
        maps.append(m)
    return maps


def assemble(results, H):
    """Unshard: stitch the exact-f32 H echo segment with the device-computed
    bf16 segments [c2q | H*c2q | H*q2c], upcast to f32."""
    H = np.asarray(H, dtype=np.float32)
    full = np.empty((B, T, 4 * D), dtype=np.float32)
    full[:, :, 0:D] = H
    # out is [BL, 128(p), NT, 3D] -> [BL, T, 3D] with t = n*128 + p
    rest = np.concatenate(
        [
            np.asarray(results[i]["out"]).transpose(0, 2, 1, 3).reshape(
                BL, T, 3 * D
            )
            for i in range(NCORES)
        ],
        axis=0,
    )
    full[:, :, D:] = rest.astype(np.float32)
    return full


def kernel(H, U, W, b):
    nc = get_nc()
    in_maps = make_in_maps(H, U, W, b)
    res = run_bass_kernel_spmd(nc, in_maps, core_ids=list(range(NCORES)))
    return assemble(res.results, H)


# revision 29
# speedup vs baseline: 1.0378x; 1.0378x over previous
"""Trainium2 Bass kernel for the BiDAF-style attention-flow layer (v2).

S[b,t,j] = H.w_h + U.w_u + (H*w_hu).U + bias
c2q      = softmax_j(S) @ U
q2c      = softmax_t(max_j S) @ H   (broadcast over t)
out      = concat([H, c2q, H*c2q, H*q2c], axis=-1)

Sharding: data-parallel over batch B=64 across 8 NeuronCores (8 batches per
core); W/b replicated; no collectives.

v2 design notes (vs the 111us v1):
 - All DRAM layouts are partition-major so every load/store is one fat
   contiguous descriptor per partition (4-12KB) instead of 0.5-2KB packets.
 - One exp activation per batch over [65, 1024]; all weight prep (sU+b,
   w_hu*U^T stationaries) is batched once up front.
 - Per-output-segment work is balanced across engines: seg0 (c2q/Z) split
   between DVE tensor_scalar and scalar activations, seg1 on DVE, seg2 on
   gpsimd with a 0-stride broadcast AP.
 - A warm-up matmul burst at kernel start keeps the PE HAM clock-gate at
   2.4GHz (v1 ran every matmul at the cold 1.2GHz rate).
 - Optional XBAR_HB mode re-creates the natural-layout H on chip from the
   d-major copy with DMA xbar transposes, saving 4.2MB/core of HBM reads.
"""

import numpy as np
import ml_dtypes

import concourse.bass as bass
import concourse.mybir as mybir
import concourse.tile as tile
from concourse.bass_utils import run_bass_kernel_spmd
from concourse.masks import make_identity

B, T, J, D = 64, 1024, 64, 256
NCORES = 8
BL = B // NCORES  # batches per core
NT = T // 128     # t-tiles per batch
F32 = mybir.dt.float32
BF16 = mybir.dt.bfloat16
AX = mybir.AxisListType.X
AF = mybir.ActivationFunctionType
MUL = mybir.AluOpType.mult
ADD = mybir.AluOpType.add
MAX = mybir.AluOpType.max

XBAR_HB = False   # build natural-layout H on chip via DMA xbar transposes
WARMUP_MM = 16    # N=512 PE warm-up matmuls (~7us cold) to flip the HAM gate
SEG0_DVE = 2      # pairs 0..SEG0_DVE-1 scale on DVE, rest on scalar ACT


def bcast(ap, n, axis=1):
    """Insert a 0-stride axis of length n into an AP (free-dim broadcast)."""
    new = list(ap.ap)
    new.insert(axis, [0, n])
    return bass.AP(tensor=ap.tensor, offset=ap.offset, ap=new)


def split_multi_waits(nc, max_waits=1):
    """Walrus in this container rejects instructions with more than a couple
    of embedded sync waits. Hoist extras into standalone EventSemaphore
    instructions right before the offending instruction."""
    n = 0
    for fn in nc.m.functions:
        for bb in fn.blocks:
            new_insts = []
            for inst in bb.instructions:
                si = getattr(inst, "sync_info", None)
                if si is not None and si.on_wait and len(si.on_wait) > max_waits:
                    waits = list(si.on_wait)
                    for w in waits[:-max_waits]:
                        n += 1
                        ev = mybir.InstEventSemaphore(
                            name=f"I-wsplit-{n}", ins=[], outs=[]
                        )
                        ev.engine = inst.engine
                        ev.sync_info = mybir.SyncInfo(on_wait=[w], on_update=[])
                        new_insts.append(ev)
                    inst.sync_info = mybir.SyncInfo(
                        on_wait=waits[-max_waits:], on_update=list(si.on_update)
                    )
                new_insts.append(inst)
            bb.instructions[:] = new_insts
    return n


def build_nc():
    nc = bass.Bass()
    HTR = nc.declare_dram_parameter("HTR", [BL, 128, 2, T], BF16, isOutput=False)
    if not XBAR_HB:
        HbR = nc.declare_dram_parameter("HbR", [BL, 128, NT, D], BF16,
                                        isOutput=False)
    Ub = nc.declare_dram_parameter("Ub", [BL, J, D], BF16, isOutput=False)
    UTb = nc.declare_dram_parameter("UTb", [128, BL, 2, J], BF16, isOutput=False)
    W = nc.declare_dram_parameter("W", [3 * D], F32, isOutput=False)
    b = nc.declare_dram_parameter("b", [1], F32, isOutput=False)
    out = nc.declare_dram_parameter("out", [BL, 128, NT, 3 * D], BF16,
                                    isOutput=True)

    with tile.TileContext(nc) as tc:
        with (
            tc.tile_pool(name="singles", bufs=1) as singles,
            tc.tile_pool(name="htp", bufs=3) as htp,
            tc.tile_pool(name="hbp", bufs=5) as hbp,
            tc.tile_pool(name="etp", bufs=2) as etp,
            tc.tile_pool(name="outp", bufs=4) as outp,
            tc.tile_pool(name="small", bufs=3) as small,
            # PSUM: 2 + 1 + 3 + 2 = 8 banks
            tc.tile_pool(name="ps_s", bufs=1, space="PSUM") as ps_s,
            tc.tile_pool(name="ps_et", bufs=1, space="PSUM") as ps_et,
            tc.tile_pool(name="ps_c", bufs=3, space="PSUM") as ps_c,
            tc.tile_pool(name="ps_q", bufs=1, space="PSUM") as ps_q,
        ):
            # ---------------- one-time setup ------------------------------
            # U/W loads go out FIRST (scalar queue) since the S stationaries
            # derive from them; the first H batches follow on sync. The sU
            # chain (ub_all, w_u_bc, b_col) loads before the rhs_w chain.
            ub_all = singles.tile([J, BL, D], BF16)
            nc.scalar.dma_start(
                out=ub_all[:], in_=Ub.rearrange("n j d -> j n d")
            )
            w_u_bc = singles.tile([J, D], F32)
            wsl = W[D : 2 * D]
            nc.scalar.dma_start(
                out=w_u_bc[:],
                in_=bass.AP(tensor=wsl.tensor, offset=wsl.offset,
                            ap=[[0, J]] + list(wsl.ap)),
            )
            b_col = singles.tile([J, 1], F32)
            bsl = b[0:1]
            nc.scalar.dma_start(
                out=b_col[:],
                in_=bass.AP(tensor=bsl.tensor, offset=bsl.offset,
                            ap=[[0, J]] + list(bsl.ap)),
            )
            ut_all = singles.tile([128, BL, 2, J], BF16)
            nc.scalar.dma_start(out=ut_all[:], in_=UTb[:])
            whu_col = singles.tile([128, 2], F32)
            wh_col = singles.tile([128, 2], F32)
            nc.scalar.dma_start(
                out=whu_col[:, :],
                in_=W[2 * D : 3 * D].rearrange("(k p) -> p k", p=128),
            )
            nc.scalar.dma_start(
                out=wh_col[:, :],
                in_=W[0:D].rearrange("(k p) -> p k", p=128),
            )

            def load_batch(bi):
                ht = htp.tile([128, 2, T], BF16, tag="ht")
                nc.sync.dma_start(out=ht[:], in_=HTR[bi])
                hb = hbp.tile([128, NT, D], BF16, tag="hb")
                if XBAR_HB:
                    for n in range(NT):
                        for c in range(2):
                            nc.sync.dma_start(
                                out=hb[:, n, 128 * c : 128 * (c + 1)],
                                in_=ht[:, c, 128 * n : 128 * (n + 1)],
                                transpose=True,
                            )
                else:
                    nc.sync.dma_start(out=hb[:], in_=HbR[bi])
                return ht, hb

            cur = load_batch(0)
            nxt = load_batch(1)

            ident_bf = singles.tile([128, 128], BF16)
            make_identity(nc, ident_bf[:])
            ones_row_bf = singles.tile([1, 128], BF16)
            nc.vector.memset(ones_row_bf[:], 1.0)
            ones_col_bf = singles.tile([128, 1], BF16)
            nc.vector.memset(ones_col_bf[:], 1.0)

            # PE warm-up: junk matmuls (N=512 via a 0-stride repeat of the
            # identity) into the S psum bank while the first DMAs land. Needs
            # >2 HAM windows (~7us) of sustained activity to reliably flip
            # the clock gate to 2.4GHz before real work starts.
            stp_warm = ps_q.tile([65, 512], F32, tag="qb")
            for i in range(WARMUP_MM):
                nc.tensor.matmul(
                    stp_warm[:], ident_bf[:, 0:65],
                    bcast(ident_bf[:], 4), start=True, stop=True,
                    skip_group_check=True,
                )

            # sU + b for all batches; row 64 stays 0 so the batched exp also
            # produces exp(sH) in its last row.
            su_scr = singles.tile([J, BL, D], F32)
            nc.gpsimd.tensor_mul(
                su_scr[:], ub_all[:], bcast(w_u_bc[:], BL)
            )
            su_raw = singles.tile([J, BL], F32)
            nc.vector.reduce_sum(su_raw[:], su_scr[:], axis=AX, op=ADD)
            su_ext = singles.tile([J + 1, BL], F32)
            nc.vector.tensor_scalar_add(su_ext[0:J, :], su_raw[:], b_col[:])
            nc.vector.memset(su_ext[J : J + 1, :], 0.0)

            # stationary weights [w_hu*U^T | w_h] for all batches
            rhs_w = singles.tile([128, BL, 2, J + 1], BF16)
            for c in range(2):
                nc.vector.tensor_scalar_mul(
                    rhs_w[:, :, c, 0:J], ut_all[:, :, c, :],
                    whu_col[:, c : c + 1],
                )
                nc.vector.tensor_copy(
                    rhs_w[:, :, c, J : J + 1],
                    bcast(wh_col[:, c : c + 1], BL),
                )

            # ---------------- per-batch pipeline --------------------------
            # The q2c tail for batch bi runs two batches later so none of
            # its six engine hops sits on the critical path.
            # pending state: [hb, ot3, em, em_s, bi, q2czt]
            pend1 = None
            pend2 = None

            def emit_q2czt(st):
                hb_, ot3_, em_, em_s_, tbi = st[:5]
                q2czt_ = ps_q.tile([1, D + 8], F32, tag="qz")
                for ti in range(NT):
                    nc.tensor.matmul(
                        q2czt_[0:1, 0:D], em_[:, ti : ti + 1], hb_[:, ti, :],
                        start=(ti == 0), stop=(ti == NT - 1),
                        skip_group_check=True,
                    )
                nc.tensor.matmul(q2czt_[0:1, D : D + 1], em_s_[:],
                                 ones_col_bf[:], start=True, stop=True,
                                 skip_group_check=True)
                st.append(q2czt_)

            def emit_tail_compute(st):
                hb_, ot3_, em_, em_s_, tbi, q2czt_ = st
                # q2c = (sum_t em*H) / (sum_t em)
                ztinv = small.tile([1, 1], F32, tag="ztinv")
                nc.vector.reciprocal(ztinv[:], q2czt_[0:1, D : D + 1])
                q2c_row = small.tile([1, D], BF16, tag="q2crow")
                nc.scalar.activation(q2c_row[:], q2czt_[0:1, 0:D], AF.Copy,
                                     scale=ztinv[:])
                q2cbp = ps_q.tile([128, D], F32, tag="qb")
                nc.tensor.matmul(q2cbp[:], ones_row_bf[:], q2c_row[:],
                                 start=True, stop=True)
                q2cb = small.tile([128, D], BF16, tag="q2cb")
                nc.scalar.copy(q2cb[:], q2cbp[:])
                # seg2 = H * q2c (broadcast over t-tiles), split DVE/gpsimd
                nc.vector.tensor_tensor(
                    out=ot3_[:, 0:4, 2 * D : 3 * D], in0=hb_[:, 0:4, :],
                    in1=bcast(q2cb[:], 4), op=MUL,
                )
                nc.gpsimd.tensor_mul(
                    ot3_[:, 4:NT, 2 * D : 3 * D], hb_[:, 4:NT, :],
                    bcast(q2cb[:], 4),
                )

            def emit_store(st):
                ot3_, tbi = st[1], st[4]
                if tbi >= BL - 2:
                    # endgame stores split in halves across both queues
                    for hh in range(2):
                        sl = slice(4 * hh, 4 * (hh + 1))
                        q = nc.sync if (hh + tbi) % 2 == 0 else nc.scalar
                        q.dma_start(out=out[tbi, :, sl], in_=ot3_[:, sl])
                else:
                    nc.sync.dma_start(out=out[tbi], in_=ot3_[:])

            for bi in range(BL):
                ht, hb = cur

                # ---- S^T: stp[j|sH, t] --------------------------------
                stp = ps_s.tile([65, 2, 512], F32, tag="stp")
                for h in range(2):
                    for c in range(2):
                        nc.tensor.matmul(
                            stp[:, h, :], rhs_w[:, bi, c, :],
                            ht[:, c, 512 * h : 512 * (h + 1)],
                            start=(c == 0), stop=(c == 1),
                        )
                    # exp half h while half 1-h is still on the PE
                    if h == 0:
                        et = etp.tile([J + 1, T], BF16, tag="et")
                    nc.scalar.activation(
                        et[:, 512 * h : 512 * (h + 1)], stp[:, h, :], AF.Exp,
                        bias=su_ext[:, bi : bi + 1], scale=1.0,
                    )

                if bi + 2 <= BL - 1:
                    cur, nxt = nxt, load_batch(bi + 2)
                else:
                    cur = nxt

                # the lag-2 batch's q2c accumulation fills the PE while the
                # scalar engine computes this batch's exp.
                if pend2 is not None:
                    emit_q2czt(pend2)

                # ---- c2q matmuls + stats transposes --------------------
                ot3 = outp.tile([128, NT, 3 * D], BF16, tag="ot")
                etT = ps_et.tile([128, NT, 80], BF16, tag="etT")
                cqs = []
                for p in range(NT // 2):
                    cq = ps_c.tile([128, 2, D], F32, tag="cq")
                    for k in range(2):
                        ti = 2 * p + k
                        nc.tensor.matmul(
                            cq[:, k, :],
                            et[0:J, 128 * ti : 128 * (ti + 1)],
                            ub_all[:, bi, :], start=True, stop=True,
                        )
                        nc.tensor.transpose(
                            etT[:, ti, 0 : J + 1],
                            et[:, 128 * ti : 128 * (ti + 1)],
                            ident_bf[0 : J + 1, 0 : J + 1],
                        )
                    cqs.append(cq)

                if pend2 is not None:
                    emit_tail_compute(pend2)

                # ---- softmax stats ------------------------------------
                r = small.tile([128, NT], BF16, tag="r")
                nc.vector.reduce_max(r[:], etT[:, :, 0:J], axis=AX, op=MAX)
                zs = small.tile([128, NT], F32, tag="zs")
                nc.vector.reduce_sum(zs[:], etT[:, :, 0:J], axis=AX, op=ADD)
                zinv = small.tile([128, NT], F32, tag="zinv")
                nc.vector.reciprocal(zinv[:], zs[:])
                # em = exp(sH) * max_j E; its per-partition sums feed the
                # deferred Zt matmul two batches later.
                em = small.tile([128, NT], BF16, tag="em")
                nc.vector.tensor_tensor(
                    out=em[:], in0=etT[:, :, J], in1=r[:], op=MUL
                )
                em_sb = small.tile([128, 1], BF16, tag="emsb")
                with nc.allow_low_precision(
                    reason="DVE reduces in f32 internally; one bf16 round"
                ):
                    nc.vector.reduce_sum(em_sb[:], em[:], axis=AX, op=ADD)

                # ---- normalize c2q (split DVE/scalar) + seg1 -----------
                for p in range(NT // 2):
                    cq = cqs[p]
                    if p < SEG0_DVE:
                        # one op per pair: zinv broadcast along d via a
                        # 0-stride axis ([128, 2, 256] * [128, 2, (0)256])
                        zsl = zinv[:, 2 * p : 2 * p + 2]
                        nc.vector.tensor_tensor(
                            out=ot3[:, 2 * p : 2 * p + 2, 0:D], in0=cq[:],
                            in1=bcast(zsl, D, axis=2), op=MUL,
                        )
                    else:
                        for k in range(2):
                            ti = 2 * p + k
                            nc.scalar.activation(
                                ot3[:, ti, 0:D], cq[:, k, :], AF.Copy,
                                scale=zinv[:, ti : ti + 1],
                            )
                # seg1 = seg0 * H in one wide bf16 op
                nc.vector.tensor_tensor(
                    out=ot3[:, :, D : 2 * D], in0=ot3[:, :, 0:D],
                    in1=hb[:], op=MUL,
                )

                if pend2 is not None:
                    emit_store(pend2)
                pend2 = pend1
                pend1 = [hb, ot3, em, em_sb, bi]

            # endgame: interleave the last two batches' tails so their
            # engine chains overlap instead of running serially.
            emit_q2czt(pend2)
            emit_q2czt(pend1)
            emit_tail_compute(pend2)
            emit_tail_compute(pend1)
            emit_store(pend2)
            emit_store(pend1)

    split_multi_waits(nc)
    return nc


_NC_CACHE = None


def get_nc():
    global _NC_CACHE
    if _NC_CACHE is None:
        _NC_CACHE = build_nc()
    return _NC_CACHE


def make_in_maps(H, U, W, b):
    """Shard + lay out inputs (host does dtype rounding + transposes only)."""
    H = np.ascontiguousarray(np.asarray(H, dtype=np.float32))
    U = np.ascontiguousarray(np.asarray(U, dtype=np.float32))
    W = np.ascontiguousarray(np.asarray(W, dtype=np.float32))
    b = np.ascontiguousarray(np.asarray(b, dtype=np.float32))
    Hb = H.astype(ml_dtypes.bfloat16)
    Ubf = U.astype(ml_dtypes.bfloat16)
    # d-major H: [B, 128(q), 2(c), T] with d = c*128 + q
    HTR = np.ascontiguousarray(
        Hb.reshape(B, T, 2, 128).transpose(0, 3, 2, 1)
    )
    # natural H, partition-major: [B, 128(p), NT, D] with t = n*128 + p
    HbR = np.ascontiguousarray(
        Hb.reshape(B, NT, 128, D).transpose(0, 2, 1, 3)
    )
    # d-major U: [128(q), B, 2(c), J]
    UTb = np.ascontiguousarray(
        Ubf.reshape(B, J, 2, 128).transpose(3, 0, 2, 1)
    )
    maps = []
    for i in range(NCORES):
        sl = slice(i * BL, (i + 1) * BL)
        m = {
            "HTR": HTR[sl],
            "Ub": Ubf[sl],
            "UTb": UTb[:, sl],
            "W": W,
            "b": b,
        }
        if not XBAR_HB:
            m["HbR"] = HbR[sl]
        maps.append(m)
    return maps


def assemble(results, H):
    """Unshard: stitch the exact-f32 H echo segment with the device-computed
    bf16 segments [c2q | H*c2q | H*q2c], upcast to f32."""
    H = np.asarray(H, dtype=np.float32)
    full = np.empty((B, T, 4 * D), dtype=np.float32)
    full[:, :, 0:D] = H
    # out is [BL, 128(p), NT, 3D] -> [BL, T, 3D] with t = n*128 + p
    rest = np.concatenate(
        [
            np.asarray(results[i]["out"]).transpose(0, 2, 1, 3).reshape(
                BL, T, 3 * D
            )
            for i in range(NCORES)
        ],
        axis=0,
    )
    full[:, :, D:] = rest.astype(np.float32)
    return full


def kernel(H, U, W, b):
    nc = get_nc()
    in_maps = make_in_maps(H, U, W, b)
    res = run_bass_kernel_spmd(nc, in_maps, core_ids=list(range(NCORES)))
    return assemble(res.results, H)
